# revision 11
# baseline (speedup 1.0000x reference)
"""Trainium2 Bass kernel for nn_AutoMemoryModule (scatter_memory).

Two-launch pipeline over 8 NeuronCores (a 2KB 8-core AllReduce measures
~55us of NRT latency — far more than a second launch, whose host
round-trip is free; each launch carries a fixed ~11.5us preamble+teardown
that neither raw bass nor fewer semaphores shrinks):

  Launch 1 (8 cores, SPMD): K-sharded first-layer matvec, the 64 MiB
    memory-bound roofline. Each core streams its 8 MiB w1 slice. To cut
    tensor-engine time ~4x vs native fp32 (4 cy/row moving operand), both
    operands are split hi+lo in bf16 on the host:
        x = xh + xl,  w = wh + wl   (all bf16; products exact in fp32 PSUM)
    Per 128-K chunk ONE matmul: stationary [xh0 xh1 xl0 xl1] (4 cols),
    moving [wh | wl] (N=128, 1 cy/row bf16), accumulating a [4, 128] PSUM
    tile whose 2x2 quadrant sum equals the fp32 h-partial. The host sums
    the 8 partial tiles and the quadrants in f64 (free).
    The w1 stream is cut into variable-size blocks (small head so the PE
    starts ~1us earlier, small tail so the last block's matmul drain is
    ~0.4us instead of ~2.7) and the block DMAs are issued round-robin from
    five engine queues so all 16 HW DMA engines engage within ~1us.
  Launch 2 (1 core): second layer + scatter/dedup/rank on one core, in a
    TRANSPOSED [128,4] layout (c4[p,k] = c_{128k+p}) that avoids all
    single-partition [1,512] row ops:
      - z4/zp4 columns via 8 exact-f32 matmuls (stationary = w2-derived
        [128,128] column block, moving = relu(h) [128,1])
      - dup-pair max / keep / validity masks as three [128,4] DVE ops
        (partner permutation, keep masks precomputed on the host from
        token VALUES; score-dependent work stays on device)
      - compare matrix rows via a DRAM bounce: c4 -> 2KB scratch -> read
        back partition-replicated as four parallel [128,128] DMAs
        (0-stride replication is only legal on the DRAM side)
      - rank_p = #(c_q > c_p): two chunks on DVE (is_gt + accum_out) and
        two on the otherwise-idle ACT engine as sign-sums
        (rank = (511 + sum sign(c_q - c_p))/2, exact for untied kept
        candidates; ACT Sign table pre-warmed at launch start)
      - device returns ranks + logits; the host applies the permutation
        and the f64 sigmoid (ranking is on logits; sigmoid is monotone)
  NOTE: engine "warmup" was tried and REGRESSED 2x — sustained activity
  power-throttles the clocks on this part; keep engines lazily busy.

Sync discipline: the toolchain allows one semaphore wait per instruction;
_split_multi_waits hoists extra waits onto same-engine NOPs.
"""
import sys
import numpy as np

sys.path.insert(0, "/opt/trn_rl_repo")

import ml_dtypes
import concourse.bass as bass
import concourse.tile as tile
from concourse import mybir
from concourse.bass_utils import run_bass_kernel_spmd
from concourse.bass import _add_dep_helper

F32 = mybir.dt.float32
BF16 = mybir.dt.bfloat16
BF = ml_dtypes.bfloat16
NEG = np.float32(-1e20)
BIG = 1.0e20
VOCAB, MSL, EMB = 32000, 256, 1024
NCORES = 8
KTOT = EMB * MSL            # 262144 per stream
KSH = KTOT // NCORES        # 32768 per core
NCHUNK = KSH // 128         # 256 matmul chunks per core
TOKS_PER_CORE = MSL // NCORES
# w1 block sizes (chunks): small head -> PE starts early; small tail ->
# short end-of-stream matmul drain. Sum must be NCHUNK.
BLOCKS = [4, 12] + [16] * 14 + [12, 4]
assert sum(BLOCKS) == NCHUNK

Alu = mybir.AluOpType
ActFn = mybir.ActivationFunctionType


def _split_multi_waits(nc):
    """This walrus build rejects instructions carrying more than one sem wait
    ("Too many sync wait commands"). Hoist all but one wait of every such
    instruction onto same-engine NOPs inserted directly before it."""
    import copy
    templates = {}
    for fn in nc.m.functions:
        for bb in fn.blocks:
            for ins in bb.instructions:
                if type(ins).__name__ == "InstEventSemaphore" \
                        and ins.engine not in templates:
                    templates[ins.engine] = ins
    n = [0]

    def make_nop(eng, w):
        tpl = templates[eng]
        nop = copy.deepcopy(tpl)
        n[0] += 1
        nop.name = f"WS-{n[0]}"
        nop.sync_info = mybir.SyncInfo(on_wait=[w], on_update=[])
        return nop

    for fn in nc.m.functions:
        for bb in fn.blocks:
            out = []
            for ins in bb.instructions:
                si = getattr(ins, "sync_info", None)
                if si is not None and si.on_wait and len(si.on_wait) > 1:
                    waits = list(si.on_wait)
                    for w in waits[:-1]:
                        out.append(make_nop(ins.engine, w))
                    si.on_wait = [waits[-1]]
                out.append(ins)
            bb.instructions[:] = out


def build_mm(split=True):
    """Launch 1: quad-split bf16 K-sharded matvec, DMA-bound."""
    nc = bass.Bass()
    hout_d = nc.dram_tensor("hout", [4, 128], F32, kind="ExternalOutput")
    # both streams ship as flat buffers of per-block contiguous
    # [128, n, 128]-layout chunks (strided DRAM reads measured ~30% slower)
    xq_d = nc.dram_tensor("xqf", [NCHUNK * 128 * 4], BF16,
                          kind="ExternalInput")
    w1f_d = nc.dram_tensor("w1f", [NCHUNK * 128 * 128], BF16,
                           kind="ExternalInput")
    with tile.TileContext(nc) as tc:
        engs = [nc.sync, nc.scalar]
        with tc.tile_pool(name="pool", bufs=1) as pool, \
             tc.tile_pool(name="psum", bufs=1, space="PSUM") as psum:
            xq = pool.tile([128, NCHUNK, 4], BF16)
            half = NCHUNK // 2 * 128 * 4
            nc.scalar.dma_start(xq[:, 0:NCHUNK // 2, :], xq_d[0:half])
            nc.sync.dma_start(xq[:, NCHUNK // 2:NCHUNK, :],
                              xq_d[half:2 * half])
            wts = []
            s = 0
            for d, n in enumerate(BLOCKS):
                wt = pool.tile([128, n, 128], BF16, tag=f"wt{d}")
                ofs = s * 128 * 128
                engs[d % len(engs)].dma_start(wt[:],
                                              w1f_d[ofs:ofs + n * 128 * 128])
                wts.append((wt, s, n))
                s += n
            ph = psum.tile([4, 128], F32)
            for (wt, s, n) in wts:
                for g in range(n):
                    c = s + g
                    nc.tensor.matmul(ph[:], xq[:, c, :], wt[:, g, :],
                                     start=(c == 0), stop=(c == NCHUNK - 1))
            hpart = pool.tile([4, 128], F32)
            nc.vector.tensor_copy(hpart[:], ph[:])
            nc.sync.dma_start(hout_d[:], hpart[:])
    if split:
        _split_multi_waits(nc)
    return nc


# pm column layout (f32): b2c4 4 | padj4 4 | keep4 4   (mask columns only;
# hh/b1 ride in the first ww chunk so the relu isn't gated on a second DMA)
PM_B2, PM_PADJ, PM_KEEP = 0, 4, 8
PM_N = 12
# ww column layout (f32): hh 1 | b1 1 | 8 x [128] z/zp blocks | ident 128
WW_HH, WW_B1, WW_BLK, WW_ID = 0, 1, 2, 2 + 8 * 128
WW_N = WW_ID + 128


def build_tail2(split=True, b2_zero=False):
    """Launch 2: transposed-layout tail on one core (see module docstring)."""
    nc = bass.Bass()
    out8_d = nc.dram_tensor("out8", [128, 8], F32, kind="ExternalOutput")
    pm_d = nc.dram_tensor("pm", [128, PM_N], F32, kind="ExternalInput")
    ww_d = nc.dram_tensor("ww", [128, WW_N], F32, kind="ExternalInput")
    with tile.TileContext(nc) as tc:
        with tc.tile_pool(name="pool", bufs=1) as pool, \
             tc.tile_pool(name="psum", bufs=1, space="PSUM") as psum:
            pm = pool.tile([128, PM_N], F32)
            nc.sync.dma_start(pm[:], pm_d[:])
            ww = pool.tile([128, WW_N], F32)
            # chunk 0 carries hh|b1|z0|zp0 so relu + first matmul start early
            nc.scalar.dma_start(ww[:, 0:258], ww_d[:, 0:258])
            nc.sync.dma_start(ww[:, WW_ID:WW_N], ww_d[:, WW_ID:WW_N])
            qengs = [nc.sync, nc.scalar, nc.sync]
            for j in range(3):
                c0 = 258 + 256 * j
                qengs[j].dma_start(ww[:, c0:c0 + 256], ww_d[:, c0:c0 + 256])
            ident = ww[:, WW_ID:WW_N]

            ones1 = pool.tile([1, 128], F32)
            nc.vector.memset(ones1[:], 1.0)
            # pre-warm the ACT Sign table while input DMAs are in flight
            warm = pool.tile([1, 1], F32, tag="warm")
            nc.vector.memset(warm[:], 1.0)
            warm2 = pool.tile([1, 1], F32, tag="warm2")
            nc.scalar.activation(warm2[:], warm[:], ActFn.Sign)

            hha = pool.tile([128, 1], F32)
            nc.vector.tensor_scalar(hha[:], ww[:, WW_HH:WW_HH + 1],
                                    ww[:, WW_B1:WW_B1 + 1], 0.0,
                                    Alu.add, Alu.max)

            z4_ps = psum.tile([128, 4], F32)
            zp4_ps = psum.tile([128, 4], F32)
            for j in range(4):
                b0 = WW_BLK + 256 * j
                nc.tensor.matmul(z4_ps[:, j:j + 1], ww[:, b0:b0 + 128],
                                 hha[:],
                                 start=True, stop=True, skip_group_check=True)
                nc.tensor.matmul(zp4_ps[:, j:j + 1],
                                 ww[:, b0 + 128:b0 + 256], hha[:],
                                 start=True, stop=True, skip_group_check=True)

            # c4 = min(max(z4 (+b2), zp4 + padj4), keep4)   [128,4] DVE ops
            padj4 = pool.tile([128, 4], F32)
            nc.vector.tensor_tensor(padj4[:], zp4_ps[:],
                                    pm[:, PM_PADJ:PM_PADJ + 4], Alu.add)
            zb4 = z4_ps
            if not b2_zero:
                zb4 = pool.tile([128, 4], F32, tag="zb4")
                nc.vector.tensor_tensor(zb4[:], z4_ps[:],
                                        pm[:, PM_B2:PM_B2 + 4], Alu.add)
            cmax4 = pool.tile([128, 4], F32)
            nc.vector.tensor_tensor(cmax4[:], zb4[:], padj4[:], Alu.max)
            c4 = pool.tile([128, 4], F32)
            nc.vector.tensor_tensor(c4[:], cmax4[:],
                                    pm[:, PM_KEEP:PM_KEEP + 4], Alu.min)
            nc4 = pool.tile([128, 4], F32)
            nc.vector.tensor_scalar(nc4[:], c4[:], -1.0, None, Alu.mult)

            # compare rows stay on-chip: one PE transpose -> ACT copy ->
            # SBUF-to-SBUF DMA into a [1,512] row -> two K=1 broadcast
            # matmuls (a DRAM-bounce broadcast measured 6.3us; per-column
            # f32 transposes + broadcasts measured 5.3us)
            tz_ps = psum.tile([4, 128], F32)
            nc.tensor.matmul(tz_ps[:], c4[:], ident,
                             start=True, stop=True, skip_group_check=True)
            tz_sb = pool.tile([4, 128], F32)
            nc.scalar.activation(tz_sb[:], tz_ps[:], ActFn.Copy)
            crow = pool.tile([1, 512], F32)
            nc.sync.dma_start(crow[0:1, :].rearrange("p (a b) -> p a b",
                                                     a=4), tz_sb[:])
            cb = psum.tile([128, 512], F32)
            for h in range(2):
                seg = slice(256 * h, 256 * (h + 1))
                nc.tensor.matmul(cb[:, seg], ones1[0:1, :], crow[0:1, seg],
                                 start=True, stop=True, skip_group_check=True)

            # ranks: 8 half-ops, DVE (is_gt counts, k even) and ACT (Sign
            # sums, k odd) in parallel; each has a private scratch so the
            # engine queues pack back-to-back
            out8 = pool.tile([128, 8], F32)
            pp = pool.tile([128, 8], F32)
            scr = []
            for kk in range(8):
                g_t = pool.tile([128, 256], F32, tag=f"G{kk}", name=f"G{kk}")
                scr.append(g_t)
            for h in range(2):
                seg = slice(256 * h, 256 * (h + 1))
                for k in range(4):
                    g = scr[2 * k + h]
                    dst = pp[:, 2 * k + h:2 * k + h + 1]
                    if k % 2 == 0:
                        nc.vector.tensor_scalar(g[:], cb[:, seg],
                                                c4[:, k:k + 1], 0.0,
                                                Alu.is_gt, Alu.add,
                                                accum_out=dst)
                    else:
                        nc.scalar.activation(g[:], cb[:, seg], ActFn.Sign,
                                             bias=nc4[:, k:k + 1], scale=1.0,
                                             accum_out=dst)
            for k in range(4):
                nc.vector.tensor_tensor(out8[:, k:k + 1],
                                        pp[:, 2 * k:2 * k + 1],
                                        pp[:, 2 * k + 1:2 * k + 2], Alu.add)
            nc.vector.tensor_copy(out8[:, 4:8], c4[:])
            nc.sync.dma_start(out8_d[:], out8[:])
    if split:
        _split_multi_waits(nc)
    return nc


_cache = {}


def _get_nc(name):
    if name not in _cache:
        _cache[name] = {
            "mm": build_mm,
            "tail": build_tail2,
            "tailz": lambda: build_tail2(b2_zero=True),
        }[name]()
    return _cache[name]


def _bfsplit(a):
    hi = a.astype(BF)
    lo = (a - hi.astype(np.float32)).astype(BF)
    return hi, lo


def _host_prep(input_tokens, memory_context, emb_table, w1, b1, w2, b2):
    it = np.asarray(input_tokens).astype(np.int64)
    mc = np.asarray(memory_context).astype(np.int64)
    emb = np.asarray(emb_table, dtype=np.float32)
    w1 = np.asarray(w1, dtype=np.float32)
    b1 = np.asarray(b1, dtype=np.float32)
    w2 = np.asarray(w2, dtype=np.float32)
    b2 = np.asarray(b2, dtype=np.float32)

    padded = np.zeros(MSL, np.int64)
    padded[:it.shape[0]] = it
    comb = np.concatenate([padded, mc])                     # [512]

    # ---- launch-2 pack ----
    b2r = np.concatenate([b2, b2]).astype(np.float32)       # [512]

    # duplicate-pair structure (token-only). Groups of size > 2 are not
    # supported by the pairwise-max tail; randint(32000) inputs of this
    # size essentially never produce them (the fixed harness input has
    # only size-2 groups).
    groups = {}
    for q in range(512):
        t = int(comb[q])
        if t != 0:
            groups.setdefault(t, []).append(q)
    assert all(len(v) <= 2 for v in groups.values()), \
        "duplicate-token group larger than 2 unsupported by this kernel"
    partner = np.full(512, -1)
    first = np.zeros(512, bool)
    for t, qs in groups.items():
        first[qs[0]] = True
        if len(qs) == 2:
            partner[qs[0]] = qs[1]
            partner[qs[1]] = qs[0]

    # wbd[j, q]: second-layer weight feeding candidate q (inp stream uses
    # hidden rows 0:64, mem stream rows 64:128); wbdp = partner-permuted
    wbd = np.zeros((128, 512), np.float32)
    wbd[0:64, 0:256] = w2
    wbd[64:128, 256:512] = w2
    wbdp = np.zeros((128, 512), np.float32)
    padjrow = np.full(512, -BIG, np.float32)
    for q in range(512):
        if partner[q] >= 0:
            wbdp[:, q] = wbd[:, partner[q]]
            padjrow[q] = b2r[partner[q]]

    def t4(row):  # [512] row -> [128,4] transposed layout
        return np.ascontiguousarray(row.reshape(4, 128).T)

    pm = np.zeros((128, PM_N), np.float32)
    pm[:, PM_B2:PM_B2 + 4] = t4(b2r)
    pm[:, PM_PADJ:PM_PADJ + 4] = t4(padjrow)
    pm[:, PM_KEEP:PM_KEEP + 4] = t4(np.where(first, BIG, -BIG)
                                    .astype(np.float32))

    # ww: hh | b1 | interleaved [z_0 zp_0 z_1 zp_1 ...] blocks | identity
    # (ww[:, WW_HH] is patched with the launch-1 partials in kernel())
    ww = np.zeros((128, WW_N), np.float32)
    ww[:, WW_B1] = np.concatenate([b1, b1])
    for j in range(4):
        ww[:, WW_BLK + 256 * j:WW_BLK + 256 * j + 128] = \
            wbd[:, 128 * j:128 * (j + 1)]
        ww[:, WW_BLK + 256 * j + 128:WW_BLK + 256 * (j + 1)] = \
            wbdp[:, 128 * j:128 * (j + 1)]
    ww[:, WW_ID:WW_N] = np.eye(128, dtype=np.float32)

    tail_common = {"pm": pm, "ww": ww, "b2_zero": not np.any(b2),
                   "comb": comb, "first": first}

    # ---- launch-1 per-core quad-split operands ----
    per_core = []
    for i in range(NCORES):
        sl = slice(TOKS_PER_CORE * i, TOKS_PER_CORE * (i + 1))
        x0 = emb[padded[sl]].reshape(NCHUNK, 128).T          # [128, 256]
        x1 = emb[mc[sl]].reshape(NCHUNK, 128).T
        xh0, xl0 = _bfsplit(x0)
        xh1, xl1 = _bfsplit(x1)
        xq = np.ascontiguousarray(
            np.stack([xh0, xh1, xl0, xl1], axis=-1))         # [128, 256, 4]
        Wc = w1[KSH * i:KSH * (i + 1)].reshape(NCHUNK, 128, 64)
        wh, wl = _bfsplit(Wc)
        whl = np.concatenate([wh, wl], axis=2)               # [256, 128, 128]
        # per-block [128, n, 128] contiguous chunks, concatenated flat
        parts = []
        s = 0
        for n in BLOCKS:
            parts.append(np.ascontiguousarray(
                whl[s:s + n].transpose(1, 0, 2)).reshape(-1))
        # xq as two contiguous [128, 128, 4] halves
            s += n
        w1f = np.concatenate(parts)
        h = NCHUNK // 2
        xqf = np.concatenate([
            np.ascontiguousarray(xq[:, 0:h, :]).reshape(-1),
            np.ascontiguousarray(xq[:, h:NCHUNK, :]).reshape(-1)])
        per_core.append({"xqf": xqf, "w1f": w1f})
    return tail_common, per_core


def _host_mid(results):
    """Sum the 8 [4,128] partials and their 2x2 quadrants (f64) -> hh[128]."""
    hq = np.zeros((4, 128), np.float64)
    for r in results:
        hq += r["hout"].astype(np.float64)
    hq2 = hq[:, 0:64] + hq[:, 64:128]                        # [4, 64]
    hh = np.concatenate([hq2[0] + hq2[2], hq2[1] + hq2[3]])  # [128]
    return hh.astype(np.float32)


def _host_post(out8, comb, first):
    """Decode device ranks + logits into the (tokens, scores) outputs."""
    rc = out8[:, 0:4].astype(np.float64)
    c4 = out8[:, 4:8].astype(np.float64)
    rank4 = np.empty((128, 4), np.float64)
    rank4[:, 0::2] = rc[:, 0::2]                 # DVE: direct #gt counts
    rank4[:, 1::2] = (511.0 + rc[:, 1::2]) / 2.  # ACT: sign-sum decode
    rankq = rank4.T.reshape(512)                 # rank of candidate q
    cq = c4.T.reshape(512)                       # logit of candidate q

    tokens = np.zeros(256, np.int32)
    scores = np.full(256, NEG, np.float32)
    used = np.zeros(256, bool)
    kept = first & (cq > -5e19)
    for q in np.nonzero(kept)[0]:
        slot = int(round(rankq[q]))
        if slot < 256:
            assert not used[slot], "device rank collision (exact f32 tie)"
            used[slot] = True
            tokens[slot] = comb[q]
            scores[slot] = np.float32(1.0 / (1.0 + np.exp(-cq[q])))
    return tokens, scores


def kernel(input_tokens, memory_context, emb_table, w1, b1, w2, b2,
           _trace=False, _tmpdir=None):
    tail_common, per_core = _host_prep(
        input_tokens, memory_context, emb_table, w1, b1, w2, b2)

    nc1 = _get_nc("mm")
    res1 = run_bass_kernel_spmd(nc1, per_core, core_ids=list(range(NCORES)),
                                trace=_trace, tmpdir=_tmpdir)
    hh = _host_mid(res1.results)

    nc2 = _get_nc("tailz" if tail_common["b2_zero"] else "tail")
    ww = tail_common["ww"].copy()
    ww[:, WW_HH] = hh
    in2 = {"pm": tail_common["pm"], "ww": ww}
    res2 = run_bass_kernel_spmd(nc2, [in2], core_ids=[0], trace=_trace)
    out8 = res2.results[0]["out8"]
    tokens, scores = _host_post(out8, tail_common["comb"],
                                tail_common["first"])
    kernel.last_result = (res1, res2)
    return tokens, scores


# revision 13
# speedup vs baseline: 1.1298x; 1.1298x over previous
"""Trainium2 Bass kernel for nn_AutoMemoryModule (scatter_memory).

Two-launch pipeline over 8 NeuronCores (a 2KB 8-core AllReduce measures
~55us of NRT latency — far more than a second launch, whose host
round-trip is free; each launch carries a fixed ~11.5us preamble+teardown
that neither raw bass nor fewer semaphores shrinks):

  Launch 1 (8 cores, SPMD): K-sharded first-layer matvec, the 64 MiB
    memory-bound roofline. Each core streams its 8 MiB w1 slice. To cut
    tensor-engine time ~4x vs native fp32 (4 cy/row moving operand), both
    operands are split hi+lo in bf16 on the host:
        x = xh + xl,  w = wh + wl   (all bf16; products exact in fp32 PSUM)
    Per 128-K chunk ONE matmul: stationary [xh0 xh1 xl0 xl1] (4 cols),
    moving [wh | wl] (N=128, 1 cy/row bf16), accumulating a [4, 128] PSUM
    tile whose 2x2 quadrant sum equals the fp32 h-partial. The host sums
    the 8 partial tiles and the quadrants in f64 (free).
    The w1 stream is cut into variable-size blocks (small head so the PE
    starts ~1us earlier, small tail so the last block's matmul drain is
    ~0.4us instead of ~2.7) and the block DMAs are issued round-robin from
    five engine queues so all 16 HW DMA engines engage within ~1us.
  Launch 2 (1 core): second layer + scatter/dedup/rank on one core, in a
    TRANSPOSED [128,4] layout (c4[p,k] = c_{128k+p}) that avoids all
    single-partition [1,512] row ops:
      - z4/zp4 columns via 8 exact-f32 matmuls (stationary = w2-derived
        [128,128] column block, moving = relu(h) [128,1])
      - dup-pair max / keep / validity masks as three [128,4] DVE ops
        (partner permutation, keep masks precomputed on the host from
        token VALUES; score-dependent work stays on device)
      - compare matrix rows via a DRAM bounce: c4 -> 2KB scratch -> read
        back partition-replicated as four parallel [128,128] DMAs
        (0-stride replication is only legal on the DRAM side)
      - rank_p = #(c_q > c_p): two chunks on DVE (is_gt + accum_out) and
        two on the otherwise-idle ACT engine as sign-sums
        (rank = (511 + sum sign(c_q - c_p))/2, exact for untied kept
        candidates; ACT Sign table pre-warmed at launch start)
      - device returns ranks + logits; the host applies the permutation
        and the f64 sigmoid (ranking is on logits; sigmoid is monotone)
  NOTE: engine "warmup" was tried and REGRESSED 2x — sustained activity
  power-throttles the clocks on this part; keep engines lazily busy.

Sync discipline: the toolchain allows one semaphore wait per instruction;
_split_multi_waits hoists extra waits onto same-engine NOPs.
"""
import sys
import numpy as np

sys.path.insert(0, "/opt/trn_rl_repo")

import ml_dtypes
import concourse.bass as bass
import concourse.tile as tile
from concourse import mybir
from concourse.bass_utils import run_bass_kernel_spmd
from concourse.bass import _add_dep_helper

F32 = mybir.dt.float32
BF16 = mybir.dt.bfloat16
BF = ml_dtypes.bfloat16
NEG = np.float32(-1e20)
BIG = 1.0e20
VOCAB, MSL, EMB = 32000, 256, 1024
NCORES = 8
KTOT = EMB * MSL            # 262144 per stream
KSH = KTOT // NCORES        # 32768 per core
NCHUNK = KSH // 128         # 256 matmul chunks per core
TOKS_PER_CORE = MSL // NCORES
# w1 block sizes (chunks): small head -> PE starts early; small tail ->
# short end-of-stream matmul drain. Sum must be NCHUNK.
BLOCKS = [4, 12] + [16] * 14 + [12, 4]
assert sum(BLOCKS) == NCHUNK

Alu = mybir.AluOpType
ActFn = mybir.ActivationFunctionType


def _split_multi_waits(nc):
    """This walrus build rejects instructions carrying more than one sem wait
    ("Too many sync wait commands"). Hoist all but one wait of every such
    instruction onto same-engine NOPs inserted directly before it."""
    import copy
    templates = {}
    for fn in nc.m.functions:
        for bb in fn.blocks:
            for ins in bb.instructions:
                if type(ins).__name__ == "InstEventSemaphore" \
                        and ins.engine not in templates:
                    templates[ins.engine] = ins
    n = [0]

    def make_nop(eng, w):
        tpl = templates[eng]
        nop = copy.deepcopy(tpl)
        n[0] += 1
        nop.name = f"WS-{n[0]}"
        nop.sync_info = mybir.SyncInfo(on_wait=[w], on_update=[])
        return nop

    for fn in nc.m.functions:
        for bb in fn.blocks:
            out = []
            for ins in bb.instructions:
                si = getattr(ins, "sync_info", None)
                if si is not None and si.on_wait and len(si.on_wait) > 1:
                    waits = list(si.on_wait)
                    for w in waits[:-1]:
                        out.append(make_nop(ins.engine, w))
                    si.on_wait = [waits[-1]]
                out.append(ins)
            bb.instructions[:] = out


def build_mm(split=True):
    """Launch 1: quad-split bf16 K-sharded matvec, DMA-bound."""
    nc = bass.Bass()
    hout_d = nc.dram_tensor("hout", [4, 128], F32, kind="ExternalOutput")
    # both streams ship as flat buffers of per-block contiguous
    # [128, n, 128]-layout chunks (strided DRAM reads measured ~30% slower)
    xq_d = nc.dram_tensor("xqf", [NCHUNK * 128 * 4], BF16,
                          kind="ExternalInput")
    w1f_d = nc.dram_tensor("w1f", [NCHUNK * 128 * 128], BF16,
                           kind="ExternalInput")
    with tile.TileContext(nc) as tc:
        engs = [nc.sync, nc.scalar]
        with tc.tile_pool(name="pool", bufs=1) as pool, \
             tc.tile_pool(name="psum", bufs=1, space="PSUM") as psum:
            xq = pool.tile([128, NCHUNK, 4], BF16)
            half = NCHUNK // 2 * 128 * 4
            nc.scalar.dma_start(xq[:, 0:NCHUNK // 2, :], xq_d[0:half])
            nc.sync.dma_start(xq[:, NCHUNK // 2:NCHUNK, :],
                              xq_d[half:2 * half])
            wts = []
            s = 0
            for d, n in enumerate(BLOCKS):
                wt = pool.tile([128, n, 128], BF16, tag=f"wt{d}")
                ofs = s * 128 * 128
                engs[d % len(engs)].dma_start(wt[:],
                                              w1f_d[ofs:ofs + n * 128 * 128])
                wts.append((wt, s, n))
                s += n
            ph = psum.tile([4, 128], F32)
            for (wt, s, n) in wts:
                for g in range(n):
                    c = s + g
                    nc.tensor.matmul(ph[:], xq[:, c, :], wt[:, g, :],
                                     start=(c == 0), stop=(c == NCHUNK - 1))
            hpart = pool.tile([4, 128], F32)
            nc.vector.tensor_copy(hpart[:], ph[:])
            nc.sync.dma_start(hout_d[:], hpart[:])
    if split:
        _split_multi_waits(nc)
    return nc


# pm column layout (f32): b2c4 4 | padj4 4 | keep4 4   (mask columns only;
# hh/b1 ride in the first ww chunk so the relu isn't gated on a second DMA)
PM_B2, PM_PADJ, PM_KEEP = 0, 4, 8
PM_N = 12
# ww column layout (f32): hh 1 | b1 1 | 8 x [128] z/zp blocks | ident 128
WW_HH, WW_B1, WW_BLK, WW_ID = 0, 1, 2, 2 + 8 * 128
WW_N = WW_ID + 128


def build_tail2(split=True, b2_zero=False):
    """Launch 2: transposed-layout tail on one core (see module docstring)."""
    nc = bass.Bass()
    out8_d = nc.dram_tensor("out8", [128, 8], F32, kind="ExternalOutput")
    pm_d = nc.dram_tensor("pm", [128, PM_N], F32, kind="ExternalInput")
    ww_d = nc.dram_tensor("ww", [128, WW_N], F32, kind="ExternalInput")
    with tile.TileContext(nc) as tc:
        with tc.tile_pool(name="pool", bufs=1) as pool, \
             tc.tile_pool(name="psum", bufs=1, space="PSUM") as psum:
            pm = pool.tile([128, PM_N], F32)
            nc.sync.dma_start(pm[:], pm_d[:])
            ww = pool.tile([128, WW_N], F32)
            # chunk 0 carries hh|b1|z0|zp0 so relu + first matmul start early
            nc.scalar.dma_start(ww[:, 0:258], ww_d[:, 0:258])
            nc.sync.dma_start(ww[:, WW_ID:WW_N], ww_d[:, WW_ID:WW_N])
            qengs = [nc.sync, nc.scalar, nc.sync]
            for j in range(3):
                c0 = 258 + 256 * j
                qengs[j].dma_start(ww[:, c0:c0 + 256], ww_d[:, c0:c0 + 256])
            ident = ww[:, WW_ID:WW_N]

            ones1 = pool.tile([1, 128], F32)
            nc.vector.memset(ones1[:], 1.0)
            # pre-warm the ACT Sign table while input DMAs are in flight
            warm = pool.tile([1, 1], F32, tag="warm")
            nc.vector.memset(warm[:], 1.0)
            warm2 = pool.tile([1, 1], F32, tag="warm2")
            nc.scalar.activation(warm2[:], warm[:], ActFn.Sign)

            hha = pool.tile([128, 1], F32)
            nc.vector.tensor_scalar(hha[:], ww[:, WW_HH:WW_HH + 1],
                                    ww[:, WW_B1:WW_B1 + 1], 0.0,
                                    Alu.add, Alu.max)

            z4_ps = psum.tile([128, 4], F32)
            zp4_ps = psum.tile([128, 4], F32)
            for j in range(4):
                b0 = WW_BLK + 256 * j
                nc.tensor.matmul(z4_ps[:, j:j + 1], ww[:, b0:b0 + 128],
                                 hha[:],
                                 start=True, stop=True, skip_group_check=True)
                nc.tensor.matmul(zp4_ps[:, j:j + 1],
                                 ww[:, b0 + 128:b0 + 256], hha[:],
                                 start=True, stop=True, skip_group_check=True)

            # c4 = min(max(z4 (+b2), zp4 + padj4), keep4)   [128,4] DVE ops
            padj4 = pool.tile([128, 4], F32)
            nc.vector.tensor_tensor(padj4[:], zp4_ps[:],
                                    pm[:, PM_PADJ:PM_PADJ + 4], Alu.add)
            zb4 = z4_ps
            if not b2_zero:
                zb4 = pool.tile([128, 4], F32, tag="zb4")
                nc.vector.tensor_tensor(zb4[:], z4_ps[:],
                                        pm[:, PM_B2:PM_B2 + 4], Alu.add)
            cmax4 = pool.tile([128, 4], F32)
            nc.vector.tensor_tensor(cmax4[:], zb4[:], padj4[:], Alu.max)
            c4 = pool.tile([128, 4], F32)
            nc.vector.tensor_tensor(c4[:], cmax4[:],
                                    pm[:, PM_KEEP:PM_KEEP + 4], Alu.min)
            nc4 = pool.tile([128, 4], F32)
            nc.vector.tensor_scalar(nc4[:], c4[:], -1.0, None, Alu.mult)

            # compare rows stay on-chip: one PE transpose -> ACT copy ->
            # SBUF-to-SBUF DMA into a [1,512] row -> two K=1 broadcast
            # matmuls (a DRAM-bounce broadcast measured 6.3us; per-column
            # f32 transposes + broadcasts measured 5.3us)
            tz_ps = psum.tile([4, 128], F32)
            nc.tensor.matmul(tz_ps[:], c4[:], ident,
                             start=True, stop=True, skip_group_check=True)
            tz_sb = pool.tile([4, 128], F32)
            nc.scalar.activation(tz_sb[:], tz_ps[:], ActFn.Copy)
            crow = pool.tile([1, 512], F32)
            nc.sync.dma_start(crow[0:1, :].rearrange("p (a b) -> p a b",
                                                     a=4), tz_sb[:])
            cb = psum.tile([128, 512], F32)
            for h in range(2):
                seg = slice(256 * h, 256 * (h + 1))
                nc.tensor.matmul(cb[:, seg], ones1[0:1, :], crow[0:1, seg],
                                 start=True, stop=True, skip_group_check=True)

            # ranks: full-width ops, DVE (is_gt counts, k even) and ACT
            # (Sign sums, k odd) in parallel; private scratch AND private
            # accumulator tiles — per-tile dependency tracking otherwise
            # serializes the two engines on the shared output tile
            out8 = pool.tile([128, 8], F32)
            rks = []
            for kk in range(4):
                r_t = pool.tile([128, 1], F32, tag=f"rk{kk}", name=f"rk{kk}")
                rks.append(r_t)
            scr = []
            for kk in range(4):
                g_t = pool.tile([128, 512], F32, tag=f"G{kk}", name=f"G{kk}")
                scr.append(g_t)
            for k in range(4):
                if k % 2 == 0:
                    nc.vector.tensor_scalar(scr[k][:], cb[:],
                                            c4[:, k:k + 1], 0.0,
                                            Alu.is_gt, Alu.add,
                                            accum_out=rks[k][:])
                else:
                    nc.scalar.activation(scr[k][:], cb[:], ActFn.Sign,
                                         bias=nc4[:, k:k + 1], scale=1.0,
                                         accum_out=rks[k][:])
            for k in range(4):
                nc.vector.tensor_copy(out8[:, k:k + 1], rks[k][:])
            nc.vector.tensor_copy(out8[:, 4:8], c4[:])
            nc.sync.dma_start(out8_d[:], out8[:])
    if split:
        _split_multi_waits(nc)
    return nc


_cache = {}


def _get_nc(name):
    if name not in _cache:
        _cache[name] = {
            "mm": build_mm,
            "tail": build_tail2,
            "tailz": lambda: build_tail2(b2_zero=True),
        }[name]()
    return _cache[name]


def _bfsplit(a):
    hi = a.astype(BF)
    lo = (a - hi.astype(np.float32)).astype(BF)
    return hi, lo


def _host_prep(input_tokens, memory_context, emb_table, w1, b1, w2, b2):
    it = np.asarray(input_tokens).astype(np.int64)
    mc = np.asarray(memory_context).astype(np.int64)
    emb = np.asarray(emb_table, dtype=np.float32)
    w1 = np.asarray(w1, dtype=np.float32)
    b1 = np.asarray(b1, dtype=np.float32)
    w2 = np.asarray(w2, dtype=np.float32)
    b2 = np.asarray(b2, dtype=np.float32)

    padded = np.zeros(MSL, np.int64)
    padded[:it.shape[0]] = it
    comb = np.concatenate([padded, mc])                     # [512]

    # ---- launch-2 pack ----
    b2r = np.concatenate([b2, b2]).astype(np.float32)       # [512]

    # duplicate-pair structure (token-only). Groups of size > 2 are not
    # supported by the pairwise-max tail; randint(32000) inputs of this
    # size essentially never produce them (the fixed harness input has
    # only size-2 groups).
    groups = {}
    for q in range(512):
        t = int(comb[q])
        if t != 0:
            groups.setdefault(t, []).append(q)
    assert all(len(v) <= 2 for v in groups.values()), \
        "duplicate-token group larger than 2 unsupported by this kernel"
    partner = np.full(512, -1)
    first = np.zeros(512, bool)
    for t, qs in groups.items():
        first[qs[0]] = True
        if len(qs) == 2:
            partner[qs[0]] = qs[1]
            partner[qs[1]] = qs[0]

    # wbd[j, q]: second-layer weight feeding candidate q (inp stream uses
    # hidden rows 0:64, mem stream rows 64:128); wbdp = partner-permuted
    wbd = np.zeros((128, 512), np.float32)
    wbd[0:64, 0:256] = w2
    wbd[64:128, 256:512] = w2
    wbdp = np.zeros((128, 512), np.float32)
    padjrow = np.full(512, -BIG, np.float32)
    for q in range(512):
        if partner[q] >= 0:
            wbdp[:, q] = wbd[:, partner[q]]
            padjrow[q] = b2r[partner[q]]

    def t4(row):  # [512] row -> [128,4] transposed layout
        return np.ascontiguousarray(row.reshape(4, 128).T)

    pm = np.zeros((128, PM_N), np.float32)
    pm[:, PM_B2:PM_B2 + 4] = t4(b2r)
    pm[:, PM_PADJ:PM_PADJ + 4] = t4(padjrow)
    pm[:, PM_KEEP:PM_KEEP + 4] = t4(np.where(first, BIG, -BIG)
                                    .astype(np.float32))

    # ww: hh | b1 | interleaved [z_0 zp_0 z_1 zp_1 ...] blocks | identity
    # (ww[:, WW_HH] is patched with the launch-1 partials in kernel())
    ww = np.zeros((128, WW_N), np.float32)
    ww[:, WW_B1] = np.concatenate([b1, b1])
    for j in range(4):
        ww[:, WW_BLK + 256 * j:WW_BLK + 256 * j + 128] = \
            wbd[:, 128 * j:128 * (j + 1)]
        ww[:, WW_BLK + 256 * j + 128:WW_BLK + 256 * (j + 1)] = \
            wbdp[:, 128 * j:128 * (j + 1)]
    ww[:, WW_ID:WW_N] = np.eye(128, dtype=np.float32)

    tail_common = {"pm": pm, "ww": ww, "b2_zero": not np.any(b2),
                   "comb": comb, "first": first}

    # ---- launch-1 per-core quad-split operands ----
    per_core = []
    for i in range(NCORES):
        sl = slice(TOKS_PER_CORE * i, TOKS_PER_CORE * (i + 1))
        x0 = emb[padded[sl]].reshape(NCHUNK, 128).T          # [128, 256]
        x1 = emb[mc[sl]].reshape(NCHUNK, 128).T
        xh0, xl0 = _bfsplit(x0)
        xh1, xl1 = _bfsplit(x1)
        xq = np.ascontiguousarray(
            np.stack([xh0, xh1, xl0, xl1], axis=-1))         # [128, 256, 4]
        Wc = w1[KSH * i:KSH * (i + 1)].reshape(NCHUNK, 128, 64)
        wh, wl = _bfsplit(Wc)
        whl = np.concatenate([wh, wl], axis=2)               # [256, 128, 128]
        # per-block [128, n, 128] contiguous chunks, concatenated flat
        parts = []
        s = 0
        for n in BLOCKS:
            parts.append(np.ascontiguousarray(
                whl[s:s + n].transpose(1, 0, 2)).reshape(-1))
        # xq as two contiguous [128, 128, 4] halves
            s += n
        w1f = np.concatenate(parts)
        h = NCHUNK // 2
        xqf = np.concatenate([
            np.ascontiguousarray(xq[:, 0:h, :]).reshape(-1),
            np.ascontiguousarray(xq[:, h:NCHUNK, :]).reshape(-1)])
        per_core.append({"xqf": xqf, "w1f": w1f})
    return tail_common, per_core


def _host_mid(results):
    """Sum the 8 [4,128] partials and their 2x2 quadrants (f64) -> hh[128]."""
    hq = np.zeros((4, 128), np.float64)
    for r in results:
        hq += r["hout"].astype(np.float64)
    hq2 = hq[:, 0:64] + hq[:, 64:128]                        # [4, 64]
    hh = np.concatenate([hq2[0] + hq2[2], hq2[1] + hq2[3]])  # [128]
    return hh.astype(np.float32)


def _host_post(out8, comb, first):
    """Decode device ranks + logits into the (tokens, scores) outputs."""
    rc = out8[:, 0:4].astype(np.float64)
    c4 = out8[:, 4:8].astype(np.float64)
    rank4 = np.empty((128, 4), np.float64)
    rank4[:, 0::2] = rc[:, 0::2]                 # DVE: direct #gt counts
    rank4[:, 1::2] = (511.0 + rc[:, 1::2]) / 2.  # ACT: sign-sum decode
    rankq = rank4.T.reshape(512)                 # rank of candidate q
    cq = c4.T.reshape(512)                       # logit of candidate q

    tokens = np.zeros(256, np.int32)
    scores = np.full(256, NEG, np.float32)
    used = np.zeros(256, bool)
    kept = first & (cq > -5e19)
    for q in np.nonzero(kept)[0]:
        slot = int(round(rankq[q]))
        if slot < 256:
            assert not used[slot], "device rank collision (exact f32 tie)"
            used[slot] = True
            tokens[slot] = comb[q]
            scores[slot] = np.float32(1.0 / (1.0 + np.exp(-cq[q])))
    return tokens, scores


def kernel(input_tokens, memory_context, emb_table, w1, b1, w2, b2,
           _trace=False, _tmpdir=None):
    tail_common, per_core = _host_prep(
        input_tokens, memory_context, emb_table, w1, b1, w2, b2)

    nc1 = _get_nc("mm")
    res1 = run_bass_kernel_spmd(nc1, per_core, core_ids=list(range(NCORES)),
                                trace=_trace, tmpdir=_tmpdir)
    hh = _host_mid(res1.results)

    nc2 = _get_nc("tailz" if tail_common["b2_zero"] else "tail")
    ww = tail_common["ww"].copy()
    ww[:, WW_HH] = hh
    in2 = {"pm": tail_common["pm"], "ww": ww}
    res2 = run_bass_kernel_spmd(nc2, [in2], core_ids=[0], trace=_trace)
    out8 = res2.results[0]["out8"]
    tokens, scores = _host_post(out8, tail_common["comb"],
                                tail_common["first"])
    kernel.last_result = (res1, res2)
    return tokens, scores


# revision 15
# speedup vs baseline: 1.1477x; 1.0158x over previous
"""Trainium2 Bass kernel for nn_AutoMemoryModule (scatter_memory).

Two-launch pipeline over 8 NeuronCores (a 2KB 8-core AllReduce measures
~55us of NRT latency — far more than a second launch, whose host
round-trip is free; each launch carries a fixed ~11.5us preamble+teardown
that neither raw bass nor fewer semaphores shrinks):

  Launch 1 (8 cores, SPMD): K-sharded first-layer matvec, the 64 MiB
    memory-bound roofline. Each core streams its 8 MiB w1 slice. To cut
    tensor-engine time ~4x vs native fp32 (4 cy/row moving operand), both
    operands are split hi+lo in bf16 on the host:
        x = xh + xl,  w = wh + wl   (all bf16; products exact in fp32 PSUM)
    Per 128-K chunk ONE matmul: stationary [xh0 xh1 xl0 xl1] (4 cols),
    moving [wh | wl] (N=128, 1 cy/row bf16), accumulating a [4, 128] PSUM
    tile whose 2x2 quadrant sum equals the fp32 h-partial. The host sums
    the 8 partial tiles and the quadrants in f64 (free).
    The w1 stream is cut into variable-size blocks (small head so the PE
    starts ~1us earlier, small tail so the last block's matmul drain is
    ~0.4us instead of ~2.7) and the block DMAs are issued round-robin from
    five engine queues so all 16 HW DMA engines engage within ~1us.
  Launch 2 (1 core): second layer + scatter/dedup/rank on one core, in a
    TRANSPOSED [128,4] layout (c4[p,k] = c_{128k+p}) that avoids all
    single-partition [1,512] row ops:
      - z4/zp4 columns via 8 exact-f32 matmuls (stationary = w2-derived
        [128,128] column block, moving = relu(h) [128,1])
      - dup-pair max / keep / validity masks as three [128,4] DVE ops
        (partner permutation, keep masks precomputed on the host from
        token VALUES; score-dependent work stays on device)
      - compare matrix rows via a DRAM bounce: c4 -> 2KB scratch -> read
        back partition-replicated as four parallel [128,128] DMAs
        (0-stride replication is only legal on the DRAM side)
      - rank_p = #(c_q > c_p): two chunks on DVE (is_gt + accum_out) and
        two on the otherwise-idle ACT engine as sign-sums
        (rank = (511 + sum sign(c_q - c_p))/2, exact for untied kept
        candidates; ACT Sign table pre-warmed at launch start)
      - device returns ranks + logits; the host applies the permutation
        and the f64 sigmoid (ranking is on logits; sigmoid is monotone)
  NOTE: engine "warmup" was tried and REGRESSED 2x — sustained activity
  power-throttles the clocks on this part; keep engines lazily busy.

Sync discipline: the toolchain allows one semaphore wait per instruction;
_split_multi_waits hoists extra waits onto same-engine NOPs.
"""
import sys
import numpy as np

sys.path.insert(0, "/opt/trn_rl_repo")

import ml_dtypes
import concourse.bass as bass
import concourse.tile as tile
from concourse import mybir
from concourse.bass_utils import run_bass_kernel_spmd
from concourse.bass import _add_dep_helper

F32 = mybir.dt.float32
BF16 = mybir.dt.bfloat16
BF = ml_dtypes.bfloat16
NEG = np.float32(-1e20)
BIG = 1.0e20
VOCAB, MSL, EMB = 32000, 256, 1024
NCORES = 8
KTOT = EMB * MSL            # 262144 per stream
KSH = KTOT // NCORES        # 32768 per core
NCHUNK = KSH // 128         # 256 matmul chunks per core
TOKS_PER_CORE = MSL // NCORES
# w1 block sizes (chunks): small head -> PE starts early; small tail ->
# short end-of-stream matmul drain. Sum must be NCHUNK.
BLOCKS = [4, 12] + [16] * 14 + [12, 4]
assert sum(BLOCKS) == NCHUNK

Alu = mybir.AluOpType
ActFn = mybir.ActivationFunctionType


def _split_multi_waits(nc):
    """This walrus build rejects instructions carrying more than one sem wait
    ("Too many sync wait commands"). Hoist all but one wait of every such
    instruction onto same-engine NOPs inserted directly before it."""
    import copy
    templates = {}
    for fn in nc.m.functions:
        for bb in fn.blocks:
            for ins in bb.instructions:
                if type(ins).__name__ == "InstEventSemaphore" \
                        and ins.engine not in templates:
                    templates[ins.engine] = ins
    n = [0]

    def make_nop(eng, w):
        tpl = templates[eng]
        nop = copy.deepcopy(tpl)
        n[0] += 1
        nop.name = f"WS-{n[0]}"
        nop.sync_info = mybir.SyncInfo(on_wait=[w], on_update=[])
        return nop

    for fn in nc.m.functions:
        for bb in fn.blocks:
            out = []
            for ins in bb.instructions:
                si = getattr(ins, "sync_info", None)
                if si is not None and si.on_wait and len(si.on_wait) > 1:
                    waits = list(si.on_wait)
                    for w in waits[:-1]:
                        out.append(make_nop(ins.engine, w))
                    si.on_wait = [waits[-1]]
                out.append(ins)
            bb.instructions[:] = out


def build_mm(split=True):
    """Launch 1: quad-split bf16 K-sharded matvec, DMA-bound."""
    nc = bass.Bass()
    hout_d = nc.dram_tensor("hout", [4, 128], F32, kind="ExternalOutput")
    # both streams ship as flat buffers of per-block contiguous
    # [128, n, 128]-layout chunks (strided DRAM reads measured ~30% slower)
    xq_d = nc.dram_tensor("xqf", [NCHUNK * 128 * 4], BF16,
                          kind="ExternalInput")
    w1f_d = nc.dram_tensor("w1f", [NCHUNK * 128 * 128], BF16,
                           kind="ExternalInput")
    with tile.TileContext(nc) as tc:
        engs = [nc.sync, nc.scalar]
        with tc.tile_pool(name="pool", bufs=1) as pool, \
             tc.tile_pool(name="psum", bufs=1, space="PSUM") as psum:
            xq = pool.tile([128, NCHUNK, 4], BF16)
            half = NCHUNK // 2 * 128 * 4
            nc.scalar.dma_start(xq[:, 0:NCHUNK // 2, :], xq_d[0:half])
            nc.sync.dma_start(xq[:, NCHUNK // 2:NCHUNK, :],
                              xq_d[half:2 * half])
            wts = []
            s = 0
            for d, n in enumerate(BLOCKS):
                wt = pool.tile([128, n, 128], BF16, tag=f"wt{d}")
                ofs = s * 128 * 128
                engs[d % len(engs)].dma_start(wt[:],
                                              w1f_d[ofs:ofs + n * 128 * 128])
                wts.append((wt, s, n))
                s += n
            ph = psum.tile([4, 128], F32)
            for (wt, s, n) in wts:
                for g in range(n):
                    c = s + g
                    nc.tensor.matmul(ph[:], xq[:, c, :], wt[:, g, :],
                                     start=(c == 0), stop=(c == NCHUNK - 1))
            hpart = pool.tile([4, 128], F32)
            nc.vector.tensor_copy(hpart[:], ph[:])
            nc.sync.dma_start(hout_d[:], hpart[:])
    if split:
        _split_multi_waits(nc)
    return nc


# pm column layout (f32): b2c4 4 | padj4 4 | keep4 4   (mask columns only;
# hh/b1 ride in the first ww chunk so the relu isn't gated on a second DMA)
PM_B2, PM_PADJ, PM_KEEP = 0, 4, 8
PM_N = 12
# ww column layout (f32): hh 1 | b1 1 | 8 x [128] z/zp blocks | ident 128
WW_HH, WW_B1, WW_BLK, WW_ID = 0, 1, 2, 2 + 8 * 128
WW_N = WW_ID + 128


def build_tail2(split=True, b2_zero=False):
    """Launch 2: transposed-layout tail on one core (see module docstring)."""
    nc = bass.Bass()
    out8_d = nc.dram_tensor("out8", [128, 8], F32, kind="ExternalOutput")
    pm_d = nc.dram_tensor("pm", [128, PM_N], F32, kind="ExternalInput")
    ww_d = nc.dram_tensor("ww", [128, WW_N], F32, kind="ExternalInput")
    with tile.TileContext(nc) as tc:
        with tc.tile_pool(name="pool", bufs=1) as pool, \
             tc.tile_pool(name="psum", bufs=1, space="PSUM") as psum:
            pm = pool.tile([128, PM_N], F32)
            nc.sync.dma_start(pm[:], pm_d[:])
            ww = pool.tile([128, WW_N], F32)
            # chunk 0 carries hh|b1|z0 so relu + the first matmul start
            # as early as possible; zp0 follows on the other queue
            nc.scalar.dma_start(ww[:, 0:130], ww_d[:, 0:130])
            nc.sync.dma_start(ww[:, 130:258], ww_d[:, 130:258])
            nc.scalar.dma_start(ww[:, WW_ID:WW_N], ww_d[:, WW_ID:WW_N])
            qengs = [nc.sync, nc.scalar, nc.sync]
            for j in range(3):
                c0 = 258 + 256 * j
                qengs[j].dma_start(ww[:, c0:c0 + 256], ww_d[:, c0:c0 + 256])
            ident = ww[:, WW_ID:WW_N]

            ones1 = pool.tile([1, 128], F32)
            nc.vector.memset(ones1[:], 1.0)
            # pre-warm the ACT Sign table while input DMAs are in flight
            warm = pool.tile([1, 1], F32, tag="warm")
            nc.vector.memset(warm[:], 1.0)
            warm2 = pool.tile([1, 1], F32, tag="warm2")
            nc.scalar.activation(warm2[:], warm[:], ActFn.Sign)

            hha = pool.tile([128, 1], F32)
            nc.vector.tensor_scalar(hha[:], ww[:, WW_HH:WW_HH + 1],
                                    ww[:, WW_B1:WW_B1 + 1], 0.0,
                                    Alu.add, Alu.max)

            z4_ps = psum.tile([128, 4], F32)
            zp4_ps = psum.tile([128, 4], F32)
            for j in range(4):
                b0 = WW_BLK + 256 * j
                nc.tensor.matmul(z4_ps[:, j:j + 1], ww[:, b0:b0 + 128],
                                 hha[:],
                                 start=True, stop=True, skip_group_check=True)
                nc.tensor.matmul(zp4_ps[:, j:j + 1],
                                 ww[:, b0 + 128:b0 + 256], hha[:],
                                 start=True, stop=True, skip_group_check=True)

            # c4 = min(max(z4 (+b2), zp4 + padj4), keep4)   [128,4] DVE ops
            padj4 = pool.tile([128, 4], F32)
            nc.vector.tensor_tensor(padj4[:], zp4_ps[:],
                                    pm[:, PM_PADJ:PM_PADJ + 4], Alu.add)
            zb4 = z4_ps
            if not b2_zero:
                zb4 = pool.tile([128, 4], F32, tag="zb4")
                nc.vector.tensor_tensor(zb4[:], z4_ps[:],
                                        pm[:, PM_B2:PM_B2 + 4], Alu.add)
            cmax4 = pool.tile([128, 4], F32)
            nc.vector.tensor_tensor(cmax4[:], zb4[:], padj4[:], Alu.max)
            c4 = pool.tile([128, 4], F32)
            nc.vector.tensor_tensor(c4[:], cmax4[:],
                                    pm[:, PM_KEEP:PM_KEEP + 4], Alu.min)
            nc4 = pool.tile([128, 4], F32)
            nc.vector.tensor_scalar(nc4[:], c4[:], -1.0, None, Alu.mult)

            # compare rows stay on-chip: one PE transpose -> ACT copy ->
            # SBUF-to-SBUF DMA into a [1,512] row -> two K=1 broadcast
            # matmuls (a DRAM-bounce broadcast measured 6.3us; per-column
            # f32 transposes + broadcasts measured 5.3us)
            tz_ps = psum.tile([4, 128], F32)
            nc.tensor.matmul(tz_ps[:], c4[:], ident,
                             start=True, stop=True, skip_group_check=True)
            tz_sb = pool.tile([4, 128], F32)
            nc.scalar.activation(tz_sb[:], tz_ps[:], ActFn.Copy)
            crow = pool.tile([1, 384], F32)
            nc.sync.dma_start(crow[0:1, :].rearrange("p (a b) -> p a b",
                                                     a=3), tz_sb[1:4, :])
            cb = psum.tile([128, 512], F32)
            # chunk 0 broadcasts straight from tz_sb row 0 (partition 0)
            # while the row DMA for chunks 1-3 is still in flight
            nc.tensor.matmul(cb[:, 0:128], ones1[0:1, :], tz_sb[0:1, :],
                             start=True, stop=True, skip_group_check=True)
            nc.tensor.matmul(cb[:, 128:512], ones1[0:1, :], crow[0:1, :],
                             start=True, stop=True, skip_group_check=True)

            # ranks: full-width ops, DVE (is_gt counts, k even) and ACT
            # (Sign sums, k odd) in parallel; private scratch AND private
            # accumulator tiles — per-tile dependency tracking otherwise
            # serializes the two engines on the shared output tile
            out8 = pool.tile([128, 8], F32)
            rks = []
            for kk in range(4):
                r_t = pool.tile([128, 1], F32, tag=f"rk{kk}", name=f"rk{kk}")
                rks.append(r_t)
            scr = []
            for kk in range(4):
                g_t = pool.tile([128, 512], F32, tag=f"G{kk}", name=f"G{kk}")
                scr.append(g_t)
            # ACT ranks read their own SBUF copy of cb: TileContext
            # serializes cross-engine readers of a PSUM tile
            cbs = pool.tile([128, 512], F32)
            nc.scalar.activation(cbs[:], cb[:], ActFn.Copy)
            for k in range(4):
                if k % 2 == 0:
                    nc.vector.tensor_scalar(scr[k][:], cb[:],
                                            c4[:, k:k + 1], 0.0,
                                            Alu.is_gt, Alu.add,
                                            accum_out=rks[k][:])
                else:
                    nc.scalar.activation(scr[k][:], cbs[:], ActFn.Sign,
                                         bias=nc4[:, k:k + 1], scale=1.0,
                                         accum_out=rks[k][:])
            for k in range(4):
                nc.vector.tensor_copy(out8[:, k:k + 1], rks[k][:])
            nc.vector.tensor_copy(out8[:, 4:8], c4[:])
            nc.sync.dma_start(out8_d[:], out8[:])
    if split:
        _split_multi_waits(nc)
    return nc


_cache = {}


def _get_nc(name):
    if name not in _cache:
        _cache[name] = {
            "mm": build_mm,
            "tail": build_tail2,
            "tailz": lambda: build_tail2(b2_zero=True),
        }[name]()
    return _cache[name]


def _bfsplit(a):
    hi = a.astype(BF)
    lo = (a - hi.astype(np.float32)).astype(BF)
    return hi, lo


def _host_prep(input_tokens, memory_context, emb_table, w1, b1, w2, b2):
    it = np.asarray(input_tokens).astype(np.int64)
    mc = np.asarray(memory_context).astype(np.int64)
    emb = np.asarray(emb_table, dtype=np.float32)
    w1 = np.asarray(w1, dtype=np.float32)
    b1 = np.asarray(b1, dtype=np.float32)
    w2 = np.asarray(w2, dtype=np.float32)
    b2 = np.asarray(b2, dtype=np.float32)

    padded = np.zeros(MSL, np.int64)
    padded[:it.shape[0]] = it
    comb = np.concatenate([padded, mc])                     # [512]

    # ---- launch-2 pack ----
    b2r = np.concatenate([b2, b2]).astype(np.float32)       # [512]

    # duplicate-pair structure (token-only). Groups of size > 2 are not
    # supported by the pairwise-max tail; randint(32000) inputs of this
    # size essentially never produce them (the fixed harness input has
    # only size-2 groups).
    groups = {}
    for q in range(512):
        t = int(comb[q])
        if t != 0:
            groups.setdefault(t, []).append(q)
    assert all(len(v) <= 2 for v in groups.values()), \
        "duplicate-token group larger than 2 unsupported by this kernel"
    partner = np.full(512, -1)
    first = np.zeros(512, bool)
    for t, qs in groups.items():
        first[qs[0]] = True
        if len(qs) == 2:
            partner[qs[0]] = qs[1]
            partner[qs[1]] = qs[0]

    # wbd[j, q]: second-layer weight feeding candidate q (inp stream uses
    # hidden rows 0:64, mem stream rows 64:128); wbdp = partner-permuted
    wbd = np.zeros((128, 512), np.float32)
    wbd[0:64, 0:256] = w2
    wbd[64:128, 256:512] = w2
    wbdp = np.zeros((128, 512), np.float32)
    padjrow = np.full(512, -BIG, np.float32)
    for q in range(512):
        if partner[q] >= 0:
            wbdp[:, q] = wbd[:, partner[q]]
            padjrow[q] = b2r[partner[q]]

    def t4(row):  # [512] row -> [128,4] transposed layout
        return np.ascontiguousarray(row.reshape(4, 128).T)

    pm = np.zeros((128, PM_N), np.float32)
    pm[:, PM_B2:PM_B2 + 4] = t4(b2r)
    pm[:, PM_PADJ:PM_PADJ + 4] = t4(padjrow)
    pm[:, PM_KEEP:PM_KEEP + 4] = t4(np.where(first, BIG, -BIG)
                                    .astype(np.float32))

    # ww: hh | b1 | interleaved [z_0 zp_0 z_1 zp_1 ...] blocks | identity
    # (ww[:, WW_HH] is patched with the launch-1 partials in kernel())
    ww = np.zeros((128, WW_N), np.float32)
    ww[:, WW_B1] = np.concatenate([b1, b1])
    for j in range(4):
        ww[:, WW_BLK + 256 * j:WW_BLK + 256 * j + 128] = \
            wbd[:, 128 * j:128 * (j + 1)]
        ww[:, WW_BLK + 256 * j + 128:WW_BLK + 256 * (j + 1)] = \
            wbdp[:, 128 * j:128 * (j + 1)]
    ww[:, WW_ID:WW_N] = np.eye(128, dtype=np.float32)

    tail_common = {"pm": pm, "ww": ww, "b2_zero": not np.any(b2),
                   "comb": comb, "first": first}

    # ---- launch-1 per-core quad-split operands ----
    per_core = []
    for i in range(NCORES):
        sl = slice(TOKS_PER_CORE * i, TOKS_PER_CORE * (i + 1))
        x0 = emb[padded[sl]].reshape(NCHUNK, 128).T          # [128, 256]
        x1 = emb[mc[sl]].reshape(NCHUNK, 128).T
        xh0, xl0 = _bfsplit(x0)
        xh1, xl1 = _bfsplit(x1)
        xq = np.ascontiguousarray(
            np.stack([xh0, xh1, xl0, xl1], axis=-1))         # [128, 256, 4]
        Wc = w1[KSH * i:KSH * (i + 1)].reshape(NCHUNK, 128, 64)
        wh, wl = _bfsplit(Wc)
        whl = np.concatenate([wh, wl], axis=2)               # [256, 128, 128]
        # per-block [128, n, 128] contiguous chunks, concatenated flat
        parts = []
        s = 0
        for n in BLOCKS:
            parts.append(np.ascontiguousarray(
                whl[s:s + n].transpose(1, 0, 2)).reshape(-1))
        # xq as two contiguous [128, 128, 4] halves
            s += n
        w1f = np.concatenate(parts)
        h = NCHUNK // 2
        xqf = np.concatenate([
            np.ascontiguousarray(xq[:, 0:h, :]).reshape(-1),
            np.ascontiguousarray(xq[:, h:NCHUNK, :]).reshape(-1)])
        per_core.append({"xqf": xqf, "w1f": w1f})
    return tail_common, per_core


def _host_mid(results):
    """Sum the 8 [4,128] partials and their 2x2 quadrants (f64) -> hh[128]."""
    hq = np.zeros((4, 128), np.float64)
    for r in results:
        hq += r["hout"].astype(np.float64)
    hq2 = hq[:, 0:64] + hq[:, 64:128]                        # [4, 64]
    hh = np.concatenate([hq2[0] + hq2[2], hq2[1] + hq2[3]])  # [128]
    return hh.astype(np.float32)


def _host_post(out8, comb, first):
    """Decode device ranks + logits into the (tokens, scores) outputs."""
    rc = out8[:, 0:4].astype(np.float64)
    c4 = out8[:, 4:8].astype(np.float64)
    rank4 = np.empty((128, 4), np.float64)
    rank4[:, 0::2] = rc[:, 0::2]                 # DVE: direct #gt counts
    rank4[:, 1::2] = (511.0 + rc[:, 1::2]) / 2.  # ACT: sign-sum decode
    rankq = rank4.T.reshape(512)                 # rank of candidate q
    cq = c4.T.reshape(512)                       # logit of candidate q

    tokens = np.zeros(256, np.int32)
    scores = np.full(256, NEG, np.float32)
    used = np.zeros(256, bool)
    kept = first & (cq > -5e19)
    for q in np.nonzero(kept)[0]:
        slot = int(round(rankq[q]))
        if slot < 256:
            assert not used[slot], "device rank collision (exact f32 tie)"
            used[slot] = True
            tokens[slot] = comb[q]
            scores[slot] = np.float32(1.0 / (1.0 + np.exp(-cq[q])))
    return tokens, scores


def kernel(input_tokens, memory_context, emb_table, w1, b1, w2, b2,
           _trace=False, _tmpdir=None):
    tail_common, per_core = _host_prep(
        input_tokens, memory_context, emb_table, w1, b1, w2, b2)

    nc1 = _get_nc("mm")
    res1 = run_bass_kernel_spmd(nc1, per_core, core_ids=list(range(NCORES)),
                                trace=_trace, tmpdir=_tmpdir)
    hh = _host_mid(res1.results)

    nc2 = _get_nc("tailz" if tail_common["b2_zero"] else "tail")
    ww = tail_common["ww"].copy()
    ww[:, WW_HH] = hh
    in2 = {"pm": tail_common["pm"], "ww": ww}
    res2 = run_bass_kernel_spmd(nc2, [in2], core_ids=[0], trace=_trace)
    out8 = res2.results[0]["out8"]
    tokens, scores = _host_post(out8, tail_common["comb"],
                                tail_common["first"])
    kernel.last_result = (res1, res2)
    return tokens, scores
